# revision 15
# baseline (speedup 1.0000x reference)
"""Trainium2 Bass kernel for a GINE message-passing layer.

Reference computation (N=100000 nodes, E=600000 edges, D=128):
    msg  = relu(x[src] + edge_attr)            # [E, D]
    aggr = segment_sum(msg, dst, N)            # [N, D]
    z    = (1 + eps) * x + aggr
    h    = relu(bn1(z @ W1.T + b1)) @ W2.T + b2
    out  = relu(bn2(x + h))

Distribution strategy (8 NeuronCores, host-side shard/unshard):
  * Nodes are partitioned across the 8 cores (graph parallel).  Node->core
    and node->window assignment is degree-balanced (LPT) so every 128-node
    "window" of each core receives a near-equal number of incoming edges.
  * Edges are assigned to the core that owns their destination node, so the
    segment-sum is core-local.  Every core keeps the full gather table
    (src-chunked copy of x) in its HBM and gathers x[src] rows with the
    SWDGE dma_gather instruction (the "halo" is read on demand - full input
    replication makes the all-gather a host-side copy).
  * dma_gather indices are int16, so the gather table is split into 4
    chunks of 25088 rows; nodes are assigned to chunks balancing summed
    src-degree, so each (window, chunk) edge group fits a fixed number of
    128-edge blocks (SPMD-uniform geometry across all cores).
  * MLP weights / BN parameters are replicated (folded into per-feature
    affine scale+bias on the host; O(D) work).

Per-core device pipeline (feature-major activations, [feat, node] tiles):
  1. dma_gather of x[src] rows (4 calls per 7-window granule, one per chunk),
  2. SWDGE accumulate-DMA adds edge_attr into the gathered tile,
  3. ScalarE relu -> messages,
  4. one-hot selection matrices S (VectorE iota-compare) turn the
     segment-sum into PE matmuls accumulated in PSUM:
         aggr[f, n] += sum_m msg[m, f] * S[m, n]
     plus an identity-matmul that adds (1+eps)*x (and transposes x to
     feature-major for free),
  5. MLP1 matmul + fused BN1+ReLU (ScalarE activation, per-partition affine),
     MLP2 matmul + identity-matmul residual + fused BN2+ReLU,
  6. PE transpose back to node-major, DMA out.
"""

import numpy as np

import concourse.bass as bass
import concourse.bacc as bacc
import concourse.mybir as mybir
import concourse.tile as tile
from concourse.bass_utils import run_bass_kernel_spmd

# ---------------------------------------------------------------- constants
N_NODES = 100000
D = 128
P = 128                      # partitions
NCORES = 8
NW = 98                      # 128-node windows per core
BPC = NW * P                 # padded nodes per core (12544)
NPAD = NCORES * BPC          # padded node table rows (100352)
WG = 1                       # windows per granule (pipeline unit)
NCHUNKS = 4                  # gather-table chunks (int16 dma_gather)
BN_EPS = 1e-5

_NC_CACHE: dict = {}
LAST_RESULTS = None          # BassKernelResults of the most recent run


# ------------------------------------------------------------- host planning
def _lpt_pack(deg_desc, nbins, cap_nodes):
    """Assign nodes (given in degree-descending order) to nbins bins of
    <=cap_nodes nodes each, greedily balancing summed degree per bin.
    Returns (bin_of_node, slot_of_node, loads)."""
    n = len(deg_desc)
    loads = np.zeros(nbins)
    cnt = np.zeros(nbins, np.int64)
    b_of = np.empty(n, np.int64)
    s_of = np.empty(n, np.int64)
    inf = np.inf
    for i in range(n):
        masked = np.where(cnt < cap_nodes, loads, inf)
        b = int(np.argmin(masked))
        b_of[i] = b
        s_of[i] = cnt[b]
        loads[b] += deg_desc[i]
        cnt[b] += 1
    return b_of, s_of, loads


def _plan(src, dst, n_nodes, ncores, nw, nchunks, ch):
    """Returns (pos_of_node, chunk_of_node, posc_of_node, kbc)."""
    bpc = nw * P
    per_core = n_nodes // ncores
    assert per_core * ncores == n_nodes and per_core <= bpc

    # --- destination side: core + window assignment by in-degree
    deg = np.bincount(dst, minlength=n_nodes)
    rank_order = np.argsort(-deg, kind="stable")
    ranks = np.arange(n_nodes)
    grp, off = divmod(ranks, ncores)
    core_of_rank = np.where(grp % 2 == 0, off, ncores - 1 - off)  # serpentine
    pos_of_node = np.empty(n_nodes, np.int64)
    deg_sorted = deg[rank_order]
    for c in range(ncores):
        m = core_of_rank == c
        w_of, s_of, _ = _lpt_pack(deg_sorted[m], nw, P)
        pos_of_node[rank_order[m]] = c * bpc + w_of * P + s_of

    # --- source side: gather-table chunk assignment by out-degree
    sdeg = np.bincount(src, minlength=n_nodes)
    sorder = np.argsort(-sdeg, kind="stable")
    c_of, s_of, _ = _lpt_pack(sdeg[sorder], nchunks, ch)
    chunk_of = np.empty(n_nodes, np.int64)
    chunk_of[sorder] = c_of
    posc_of = np.empty(n_nodes, np.int64)
    posc_of[sorder] = s_of

    # blocks per (window, chunk)
    cnt = np.bincount((pos_of_node[dst] // P) * nchunks + chunk_of[src],
                      minlength=ncores * nw * nchunks)
    kbc = max(2, int(np.ceil(cnt.max() / P)))
    return pos_of_node, chunk_of, posc_of, kbc


# ------------------------------------------------------------- device build
def _build(nw, wg, kbc, nchunks, ch, npad):
    """Build the per-core Bass program. All cores run this same program on
    different data."""
    ng = nw // wg                # granules
    cpc = wg * kbc               # gx columns per (granule, chunk)
    cpg = nchunks * cpc          # gx columns per granule
    nbc = ng * cpg               # block-columns per core
    s16 = cpc * P // 16          # idx int16 cols per gather call (=16)
    s16p = 32                    # padded slot (64B-aligned call slices)
    nidx = ng * nchunks * s16p
    f32 = mybir.dt.float32

    nc = bacc.Bacc(None)
    xg = nc.dram_tensor("xg", [npad, D], f32, kind="ExternalInput")
    ea = nc.dram_tensor("ea", [P, nbc * D], f32, kind="ExternalInput")
    xo = nc.dram_tensor("xo", [P, nw * D], f32, kind="ExternalInput")
    idx = nc.dram_tensor("idx", [P, nidx], mybir.dt.int16,
                         kind="ExternalInput")
    dstrel = nc.dram_tensor("dstrel", [P, nbc], f32, kind="ExternalInput")
    iotac = nc.dram_tensor("iotac", [P, P], f32, kind="ExternalInput")
    ideps = nc.dram_tensor("ideps", [P, P], f32, kind="ExternalInput")
    iden = nc.dram_tensor("iden", [P, P], f32, kind="ExternalInput")
    w1t = nc.dram_tensor("w1t", [D, D], f32, kind="ExternalInput")
    w2t = nc.dram_tensor("w2t", [D, D], f32, kind="ExternalInput")
    ab1 = nc.dram_tensor("ab1", [D, 2], f32, kind="ExternalInput")
    ab2 = nc.dram_tensor("ab2", [D, 2], f32, kind="ExternalInput")
    out = nc.dram_tensor("out", [P, nw * D], f32, kind="ExternalOutput")

    relu = mybir.ActivationFunctionType.Relu
    addop = mybir.AluOpType.add
    iseq = mybir.AluOpType.is_equal
    ts = bass.ts

    with tile.TileContext(nc) as tc:
        with (
            tc.tile_pool(name="const", bufs=1) as cp,
            tc.tile_pool(name="gx", bufs=4) as gxp,
            tc.tile_pool(name="ea", bufs=2) as eap,
            tc.tile_pool(name="sel", bufs=2) as sp,
            tc.tile_pool(name="xot", bufs=2) as xop,
            tc.tile_pool(name="z", bufs=4) as zp,
            tc.tile_pool(name="u", bufs=4) as up,
            tc.tile_pool(name="ofm", bufs=4) as ofp,
            tc.tile_pool(name="osb", bufs=2) as osp,
            tc.tile_pool(name="pz", bufs=2, space="PSUM") as pzp,
            tc.tile_pool(name="ph", bufs=2, space="PSUM") as php,
            tc.tile_pool(name="p2", bufs=2, space="PSUM") as p2p,
            tc.tile_pool(name="pt", bufs=2, space="PSUM") as ptp,
        ):
            # resident tensors
            idx_t = cp.tile([P, nidx], mybir.dt.int16)
            nc.sync.dma_start(out=idx_t[:, :], in_=idx[:, :])
            dst_t = cp.tile([P, nbc], f32)
            nc.sync.dma_start(out=dst_t[:, :], in_=dstrel[:, :])
            iota_t = cp.tile([P, P], f32)
            nc.sync.dma_start(out=iota_t[:, :], in_=iotac[:, :])
            ideps_t = cp.tile([P, P], f32)
            nc.sync.dma_start(out=ideps_t[:, :], in_=ideps[:, :])
            iden_t = cp.tile([P, P], f32)
            nc.sync.dma_start(out=iden_t[:, :], in_=iden[:, :])
            w1t_t = cp.tile([D, D], f32)
            nc.sync.dma_start(out=w1t_t[:, :], in_=w1t[:, :])
            w2t_t = cp.tile([D, D], f32)
            nc.sync.dma_start(out=w2t_t[:, :], in_=w2t[:, :])
            ab1_t = cp.tile([D, 2], f32)
            nc.sync.dma_start(out=ab1_t[:, :], in_=ab1[:, :])
            ab2_t = cp.tile([D, 2], f32)
            nc.sync.dma_start(out=ab2_t[:, :], in_=ab2[:, :])

            for g in range(ng):
                # ---- messages: gather x[src] per chunk, += edge_attr, relu
                # one 256-index dma_gather per (granule, chunk); the
                # 4-deep tile pool bounds in-flight SWDGE descriptors to
                # 4*256 = 1024 (the dynamic-DMA ring capacity - more, or
                # >1024 indices in one call, crashes the device).
                # edge_attr streams via HWDGE and is added on VectorE.
                ea_t = eap.tile([P, cpg * D], f32)
                nc.sync.dma_start(
                    out=ea_t[:, :],
                    in_=ea[:, g * cpg * D:(g + 1) * cpg * D],
                )
                gxts = []
                for c in range(nchunks):
                    gxc = gxp.tile([P, cpc * D], f32, tag="gxc")
                    nc.gpsimd.dma_gather(
                        gxc[:, :].rearrange("p (k d) -> p k d", d=D),
                        xg[c * ch:(c + 1) * ch, :],
                        idx_t[:, (g * nchunks + c) * s16p:
                              (g * nchunks + c) * s16p + s16],
                        cpc * P,
                        cpc * P,
                        D,
                    )
                    nc.vector.tensor_add(
                        out=gxc[:, :], in0=gxc[:, :],
                        in1=ea_t[:, c * cpc * D:(c + 1) * cpc * D])
                    nc.scalar.activation(
                        out=gxc[:, :], in_=gxc[:, :], func=relu)
                    gxts.append(gxc)

                # ---- own nodes (node-major x rows for this granule)
                xot = xop.tile([P, wg * D], f32)
                nc.sync.dma_start(
                    out=xot[:, :], in_=xo[:, g * wg * D:(g + 1) * wg * D]
                )

                # ---- one-hot selection matrices for the whole granule
                sel = sp.tile([P, cpg * D], f32)
                for c in range(nchunks):
                    col0 = g * cpg + c * cpc
                    in0 = (
                        dst_t[:, col0:col0 + cpc]
                        .rearrange("p (j o) -> p j o", o=1)
                        .to_broadcast([P, cpc, P])
                    )
                    in1 = (
                        iota_t[:, :]
                        .rearrange("p (o n) -> p o n", o=1)
                        .to_broadcast([P, cpc, P])
                    )
                    # The S3S3D3_TT ISA struct only holds ONE sync wait;
                    # Bacc.compile()'s generate_event_semaphores splits any
                    # excess waits into standalone EventSemaphore ops.
                    nc.vector.tensor_tensor(
                        out=sel[:, c * cpc * D:(c + 1) * cpc * D]
                        .rearrange("p (j n) -> p j n", j=cpc),
                        in0=in0,
                        in1=in1,
                        op=iseq,
                    )

                osb = osp.tile([P, wg * D], f32)
                for wi in range(wg):
                    # ---- aggregation: z[f,n] = (1+eps)x + sum(msg) in PSUM
                    pz = pzp.tile([P, P], f32, space="PSUM")
                    mm = 0
                    for c in range(nchunks):
                        for b in range(kbc):
                            col = c * cpc + wi * kbc + b
                            nc.tensor.matmul(
                                out=pz[:, :],
                                lhsT=gxts[c][:, ts(wi * kbc + b, D)],
                                rhs=sel[:, ts(col, D)],
                                start=(mm == 0),
                                stop=False,
                            )
                            mm += 1
                    nc.tensor.matmul(
                        out=pz[:, :],
                        lhsT=xot[:, ts(wi, D)],
                        rhs=ideps_t[:, :],
                        start=False,
                        stop=True,
                    )
                    z = zp.tile([P, P], f32)
                    nc.vector.tensor_copy(out=z[:, :], in_=pz[:, :])

                    # ---- MLP layer 1 + BN1 + relu
                    ph = php.tile([P, P], f32, space="PSUM")
                    nc.tensor.matmul(
                        out=ph[:, :], lhsT=w1t_t[:, :], rhs=z[:, :],
                        start=True, stop=True,
                    )
                    u = up.tile([P, P], f32)
                    nc.scalar.activation(
                        out=u[:, :], in_=ph[:, :], func=relu,
                        scale=ab1_t[:, 0:1], bias=ab1_t[:, 1:2],
                    )

                    # ---- MLP layer 2 + residual + BN2 + relu
                    p2 = p2p.tile([P, P], f32, space="PSUM")
                    nc.tensor.matmul(
                        out=p2[:, :], lhsT=w2t_t[:, :], rhs=u[:, :],
                        start=True, stop=False,
                    )
                    nc.tensor.matmul(
                        out=p2[:, :], lhsT=xot[:, ts(wi, D)],
                        rhs=iden_t[:, :], start=False, stop=True,
                    )
                    ofm = ofp.tile([P, P], f32)
                    nc.scalar.activation(
                        out=ofm[:, :], in_=p2[:, :], func=relu,
                        scale=ab2_t[:, 0:1], bias=ab2_t[:, 1:2],
                    )

                    # ---- back to node-major
                    pt = ptp.tile([P, P], f32, space="PSUM")
                    nc.tensor.transpose(
                        out=pt[:, :], in_=ofm[:, :], identity=iden_t[:, :]
                    )
                    nc.vector.tensor_copy(out=osb[:, ts(wi, D)], in_=pt[:, :])

                nc.sync.dma_start(
                    out=out[:, g * wg * D:(g + 1) * wg * D], in_=osb[:, :]
                )

    nc.compile()
    return nc


def _get_nc(key):
    if key not in _NC_CACHE:
        _NC_CACHE[key] = _build(*key)
    return _NC_CACHE[key]


# --------------------------------------------------------------- host driver
def _prepare(x, edge_index, edge_attr, eps, W1, b1, g1, bt1, rm1, rv1,
             W2, b2, g2, bt2, rm2, rv2, n_nodes, ncores, nw, wg, nchunks):
    """Shard + reformat all inputs.
    Returns (in_maps, kbc, ch, pos_of_node)."""
    bpc = nw * P
    npad = ncores * bpc
    ch = npad // nchunks
    src = np.asarray(edge_index[0], dtype=np.int64)
    dst = np.asarray(edge_index[1], dtype=np.int64)
    e = len(src)

    pos_of_node, chunk_of, posc_of, kbc = _plan(
        src, dst, n_nodes, ncores, nw, nchunks, ch)

    ng = nw // wg
    cpc = wg * kbc
    cpg = nchunks * cpc
    nbc = ng * cpg

    # --- edge -> slot: group by (core-window, chunk), pad to kbc blocks
    src_p = pos_of_node[src]
    dst_p = pos_of_node[dst]
    wgid = dst_p // P                       # global window id
    cid = chunk_of[src]
    gid = wgid * nchunks + cid              # (window, chunk) group
    order = np.argsort(gid, kind="stable")
    counts = np.bincount(gid, minlength=ncores * nw * nchunks)
    assert counts.max() <= kbc * P, (counts.max(), kbc * P)
    starts = np.zeros(ncores * nw * nchunks, np.int64)
    np.cumsum(counts[:-1], out=starts[1:])
    offs = np.arange(e, dtype=np.int64) - starts[gid[order]]

    # slot -> (global column, partition): group g=(core,w,c) occupies kbc
    # columns; within core, col = gg*cpg + c*cpc + wi*kbc + b
    og = gid[order]
    core_o = og // (nw * nchunks)
    w_o = (og // nchunks) % nw
    c_o = og % nchunks
    gg_o, wi_o = np.divmod(w_o, wg)
    col = (core_o * nbc + gg_o * cpg + c_o * cpc + wi_o * kbc + offs // P)
    prt = offs % P

    tot_cols = ncores * nbc
    srcidx_full = np.zeros((tot_cols, P), np.int16)
    srcidx_full[col, prt] = posc_of[src[order]].astype(np.int16)
    dstrel_full = np.full((tot_cols, P), -1.0, np.float32)
    dstrel_full[col, prt] = (dst_p[order] % P).astype(np.float32)
    ea_full = np.zeros((tot_cols, P, D), np.float32)
    ea_full[col, prt] = np.asarray(edge_attr, dtype=np.float32)[order]

    # --- device layouts
    # idx: per call (g,c): flat n = col_in_call*128+p ; sbuf[p, s] =
    # flat[s*16 + p%16], replicated over partition groups of 16
    s16 = cpc * P // 16
    s16p = 32
    F = srcidx_full.reshape(ncores, ng, nchunks, cpc * P)
    W_ = F.reshape(ncores, ng, nchunks, s16, 16)
    pmod = np.arange(P) % 16
    wrapped = W_[:, :, :, :, pmod].transpose(0, 4, 1, 2, 3)
    idx_dev = np.zeros((ncores, P, ng, nchunks, s16p), np.int16)
    idx_dev[:, :, :, :, :s16] = wrapped
    idx_dev = np.ascontiguousarray(
        idx_dev.reshape(ncores, P, ng * nchunks * s16p))

    dstrel_c = np.ascontiguousarray(
        dstrel_full.reshape(ncores, nbc, P).transpose(0, 2, 1))
    ea_c = np.ascontiguousarray(
        ea_full.reshape(ncores, nbc, P, D).transpose(0, 2, 1, 3)
        .reshape(ncores, P, nbc * D))

    # gather table (chunk-major, by src position)
    xg = np.zeros((npad, D), np.float32)
    xg[chunk_of * ch + posc_of] = np.asarray(x, dtype=np.float32)

    # own-node rows (window-major, by dst position)
    xperm = np.zeros((npad, D), np.float32)
    xperm[pos_of_node] = np.asarray(x, dtype=np.float32)
    xo_c = np.ascontiguousarray(
        xperm.reshape(ncores, nw, P, D).transpose(0, 2, 1, 3)
        .reshape(ncores, P, nw * D))

    # --- replicated constants
    epsf = float(np.asarray(eps))
    iotac = np.tile(np.arange(P, dtype=np.float32), (P, 1))
    ideps = ((1.0 + epsf) * np.eye(P)).astype(np.float32)
    iden = np.eye(P, dtype=np.float32)
    w1tm = np.ascontiguousarray(np.asarray(W1, np.float32).T)
    w2tm = np.ascontiguousarray(np.asarray(W2, np.float32).T)
    inv1 = 1.0 / np.sqrt(np.asarray(rv1, np.float32) + BN_EPS)
    a1 = np.asarray(g1, np.float32) * inv1
    beta1 = a1 * np.asarray(b1, np.float32) + np.asarray(bt1, np.float32) \
        - np.asarray(rm1, np.float32) * a1
    inv2 = 1.0 / np.sqrt(np.asarray(rv2, np.float32) + BN_EPS)
    a2 = np.asarray(g2, np.float32) * inv2
    beta2 = a2 * np.asarray(b2, np.float32) + np.asarray(bt2, np.float32) \
        - np.asarray(rm2, np.float32) * a2
    ab1 = np.ascontiguousarray(np.stack([a1, beta1], 1).astype(np.float32))
    ab2 = np.ascontiguousarray(np.stack([a2, beta2], 1).astype(np.float32))

    in_maps = []
    for c in range(ncores):
        in_maps.append({
            "xg": xg,
            "ea": ea_c[c],
            "xo": xo_c[c],
            "idx": idx_dev[c],
            "dstrel": dstrel_c[c],
            "iotac": iotac,
            "ideps": ideps,
            "iden": iden,
            "w1t": w1tm,
            "w2t": w2tm,
            "ab1": ab1,
            "ab2": ab2,
        })
    return in_maps, kbc, ch, pos_of_node


def kernel(**inputs) -> np.ndarray:
    global LAST_RESULTS
    x = np.asarray(inputs["x"], dtype=np.float32)
    n_nodes = x.shape[0]
    assert n_nodes == N_NODES and x.shape[1] == D

    in_maps, kbc, ch, pos_of_node = _prepare(
        x, inputs["edge_index"], inputs["edge_attr_emb"], inputs["eps"],
        inputs["W1"], inputs["b1"], inputs["g1"], inputs["bt1"],
        inputs["rm1"], inputs["rv1"],
        inputs["W2"], inputs["b2"], inputs["g2"], inputs["bt2"],
        inputs["rm2"], inputs["rv2"],
        n_nodes, NCORES, NW, WG, NCHUNKS,
    )
    nc = _get_nc((NW, WG, kbc, NCHUNKS, ch, NPAD))
    res = run_bass_kernel_spmd(nc, in_maps, core_ids=list(range(NCORES)))
    LAST_RESULTS = res

    # out[c] is [P, NW*D] partition-major; slot (p, w*D + f) = padded node
    # row c*BPC + w*P + p
    outp = np.stack([res.results[c]["out"] for c in range(NCORES)])
    out_rows = outp.reshape(NCORES, P, NW, D).transpose(0, 2, 1, 3) \
        .reshape(NPAD, D)
    return np.ascontiguousarray(out_rows[pos_of_node])
